# revision 7
# baseline (speedup 1.0000x reference)
"""Data2VecVision self-attention Bass kernel for 8 Trainium2 NeuronCores.

Sharding: data-parallel over batch (64 = 8 cores x 8 batches/core).

Per-core design (v2 — fp8 DoubleRow Q/K projections):
  - hidden_states shard transposed on host to hsT [768, 8*197]; two SBUF
    copies: fp16 (V projection) and fp8e4 pair-packed (Q/K projections).
  - Q/K projections run as N_DR DoubleRow fp8 matmuls (K=256 logical per
    pass; measured 2x fp16 throughput at FD=394) plus (6-2*N_DR) fp16
    matmuls. hs scaled x16, Wq/Wk x1024 into e4m3; fp16 remainder chunks
    scaled x2^14 so one uniform descale applies: folded into the PSUM->
    SBUF copies (DVE tensor_scalar dual-op for Q with bq add, ACT
    activation-Copy with scale for K).
  - V stays fp16 ([s, d_out] natural layout, ones column for softmax
    sums, bv kept in V via the softmax identity).
  - scores computed transposed [j, i]; head pairs at partitions 0-63 /
    64-127 run as concurrent row-group matmuls (measured dt_start 3-7ns).
  - relative-position bias folded as exp(s+b) = exp(s)*exp(b): ACT exp
    from PSUM, host-baked exp(bias) multiply on DVE (3/4) / GpSimd (1/4).
  - context for 3 head-pairs accumulates into one 1-bank PSUM tile
    [128, 390]; normalization = DVE reciprocal of sums columns + one
    broadcast multiply -> fp16 output staging; y stored fp16 (host casts
    back to fp32).
  - V-projection matmul groups interleave into the attention stream as
    PE gap fillers (lead-1 over rotated batch order).
  - input DMAs: few large transfers ordered so the first Q matmul
    unblocks after ~0.4 MB.
"""

import numpy as np
import ml_dtypes

import concourse.bacc as bacc
import concourse.mybir as mybir
import concourse.tile as tile
from concourse.bass_utils import run_bass_kernel_spmd

F32 = mybir.dt.float32
F16 = mybir.dt.float16
F8 = mybir.dt.float8e4
AF = mybir.ActivationFunctionType
ALU = mybir.AluOpType
DR = mybir.MatmulPerfMode.DoubleRow

N_CORES = 8
B = 64
NB = B // N_CORES          # batches per core
S = 197
HID = 768
HEADS = 12
D = 64
NHP = HEADS // 2           # head pairs
NCH = HID // 128           # 6 contraction chunks (fp16 view)
N_DR = 3                   # fp8 DoubleRow passes (256 hid dims each)
NF16 = NCH - 2 * N_DR      # remaining fp16 contraction chunks for Q/K
NST = 4                    # projection s-tiles per core
SW = NB * S // NST         # 394, projection moving width
CORE_S = NB * S            # 1576
JC = [(0, 128), (128, 69)]   # j/i chunk (offset, len)

SC_HS = 16.0               # fp8 scale for hidden states
SC_W = 1024.0              # fp8 scale for Wq/Wk
SC_F16 = SC_HS * SC_W      # fp16 Q/K weight chunks pre-scaled by this
DSC_Q = 1.0 / (SC_F16 * 8.0)   # descale + 1/sqrt(64)
DSC_K = 1.0 / SC_F16


def _relative_position_index(h, w):
    coords = np.stack(np.meshgrid(np.arange(h), np.arange(w), indexing="ij")).reshape(2, -1)
    rel = coords[:, :, None] - coords[:, None, :]
    rel = rel.transpose(1, 2, 0).astype(np.int64)
    rel[:, :, 0] += h - 1
    rel[:, :, 1] += w - 1
    rel[:, :, 0] *= 2 * w - 1
    area = h * w
    nrd = (2 * h - 1) * (2 * w - 1) + 3
    idx = np.zeros((area + 1, area + 1), dtype=np.int64)
    idx[1:, 1:] = rel.sum(-1)
    idx[0, :] = nrd - 3
    idx[:, 0] = nrd - 2
    idx[0, 0] = nrd - 1
    return idx


def build_nc(reps=1):
    nc = bacc.Bacc("TRN2", target_bir_lowering=False, debug=False)

    hsT_d = nc.dram_tensor("hsT", [NCH, 128, CORE_S], F16, kind="ExternalInput").ap()
    hs8_d = nc.dram_tensor("hs8", [N_DR, 128, 2 * CORE_S], F8, kind="ExternalInput").ap()
    wq8_d = nc.dram_tensor("wq8", [NCH * N_DR, 128, 256], F8, kind="ExternalInput").ap()
    wk8_d = nc.dram_tensor("wk8", [NCH * N_DR, 128, 256], F8, kind="ExternalInput").ap()
    if NF16:
        wq16_d = nc.dram_tensor("wq16", [NCH * NF16, 128, 128], F16, kind="ExternalInput").ap()
        wk16_d = nc.dram_tensor("wk16", [NCH * NF16, 128, 128], F16, kind="ExternalInput").ap()
    wv_d = nc.dram_tensor("wvT", [NCH, 128, HID], F16, kind="ExternalInput").ap()
    bq_d = nc.dram_tensor("bqc", [NCH, 128, 1], F32, kind="ExternalInput").ap()
    bv_d = nc.dram_tensor("bvb", [128, HID], F16, kind="ExternalInput").ap()
    eb_d = nc.dram_tensor("expb", [HEADS * 2, 128, S], F16, kind="ExternalInput").ap()
    y_d = nc.dram_tensor("y", [NB, S, HID], F16, kind="ExternalOutput").ap()

    with tile.TileContext(nc) as tc:
        with (
            tc.tile_pool(name="res", bufs=1) as res,
            tc.tile_pool(name="vpad", bufs=NB * 2) as vpad_pool,
            tc.tile_pool(name="et", bufs=10) as et_pool,
            tc.tile_pool(name="em", bufs=8) as em_pool,
            tc.tile_pool(name="rt", bufs=6) as rt_pool,
            tc.tile_pool(name="ot", bufs=6) as ot_pool,
            tc.tile_pool(name="pc", bufs=2, space="PSUM") as pc_ps,
            tc.tile_pool(name="sp", bufs=6, space="PSUM") as sc_ps,
        ):
            hs_sb = res.tile([128, NCH * CORE_S], F16)
            hs8_sb = res.tile([128, N_DR * 2 * CORE_S], F8)
            wq8_sb = res.tile([128, NCH * N_DR * 256], F8)
            wk8_sb = res.tile([128, NCH * N_DR * 256], F8)
            if NF16:
                wq16_sb = res.tile([128, NCH * NF16 * 128], F16)
                wk16_sb = res.tile([128, NCH * NF16 * 128], F16)
            wv_sb = res.tile([128, NCH * HID], F16)
            bq_sb = res.tile([128, NCH], F32)
            bv_sb = res.tile([128, HID], F16)
            eb_sb = res.tile([128, HEADS * 2 * S], F16)
            qt_sb = res.tile([128, NCH * CORE_S], F16)
            kt_sb = res.tile([128, NCH * CORE_S + 64], F16)
            nc.vector.memset(kt_sb[:, NCH * CORE_S:], 0.0)
            vpad = [[vpad_pool.tile([128, HEADS * 65], F16, tag="vp",
                                    name=f"vpad_{b}_{j}") for j in range(2)]
                    for b in range(NB)]

            for _ in range(reps):
                # ---- input DMAs (ordered so the first Q matmuls unblock early) ----
                dma_engs = [nc.sync, nc.scalar, nc.gpsimd]
                def dma(i, dst, src):
                    dma_engs[i % 3].dma_start(dst, src)
                # first Q (st=0) needs wq8 + hs8 st0 slices + bq
                hs8_v = hs8_sb.rearrange("p (h2 two s) -> p h2 two s",
                                         h2=N_DR, two=2)
                dma(0, wq8_sb.rearrange("p (x e) -> p x e", e=256), wq8_d.rearrange("x p e -> p x e"))
                for h2 in range(N_DR):
                    dma(1 + h2, hs8_v[:, h2, :, 0:SW],
                        hs8_d[h2].rearrange("p (two s) -> p two s", two=2)[:, :, 0:SW])
                dma(1, bq_sb[:], bq_d[:, :, 0].rearrange("c p -> p c"))
                if NF16:
                    dma(2, wq16_sb.rearrange("p (x e) -> p x e", e=128), wq16_d.rearrange("x p e -> p x e"))
                    dma(0, wk16_sb.rearrange("p (x e) -> p x e", e=128), wk16_d.rearrange("x p e -> p x e"))
                dma(2, wk8_sb.rearrange("p (x e) -> p x e", e=256), wk8_d.rearrange("x p e -> p x e"))
                for st in range(1, NST):
                    for h2 in range(N_DR):
                        dma(st + h2, hs8_v[:, h2, :, st * SW:(st + 1) * SW],
                            hs8_d[h2].rearrange("p (two s) -> p two s", two=2)
                            [:, :, st * SW:(st + 1) * SW])
                # V inputs: first batches of attention order (6,7) first
                for c in range(NCH):
                    dma(c, hs_sb[:, c * CORE_S + 6 * S: (c + 1) * CORE_S],
                        hsT_d[c, :, 6 * S:])
                dma(0, wv_sb.rearrange("p (x e) -> p x e", e=768), wv_d.rearrange("x p e -> p x e"))
                dma(1, bv_sb[:], bv_d[:])
                for c in range(NCH):
                    dma(c + 1, hs_sb[:, c * CORE_S: c * CORE_S + 6 * S],
                        hsT_d[c, :, : 6 * S])
                dma(0, eb_sb.rearrange("p (x e) -> p x e", e=197), eb_d.rearrange("x p e -> p x e"))

                # ---- Q/K projections, whole core ----
                def qk_mms(dst_ps, w8_sb, w16_sb, c, st):
                    for h2 in range(N_DR):
                        nc.tensor.matmul(
                            dst_ps[:],
                            w8_sb[:, (c * N_DR + h2) * 256:(c * N_DR + h2 + 1) * 256]
                            .rearrange("p (two m) -> p two m", two=2),
                            hs8_v[:, h2, :, st * SW:(st + 1) * SW],
                            start=(h2 == 0), stop=(h2 == N_DR - 1 and NF16 == 0),
                            perf_mode=DR)
                    for f in range(NF16):
                        hch = 2 * N_DR + f
                        nc.tensor.matmul(
                            dst_ps[:],
                            w16_sb[:, (c * NF16 + f) * 128:(c * NF16 + f + 1) * 128],
                            hs_sb[:, hch * CORE_S + st * SW: hch * CORE_S + (st + 1) * SW],
                            start=False, stop=(f == NF16 - 1))

                for st in range(NST):
                    for c in range(NCH):
                        qp = pc_ps.tile([128, SW], F32, tag="pc")
                        qk_mms(qp, wq8_sb, wq16_sb if NF16 else None, c, st)
                        nc.vector.tensor_scalar(
                            qt_sb[:, c * CORE_S + st * SW: c * CORE_S + (st + 1) * SW],
                            qp[:], DSC_Q, bq_sb[:, c:c + 1],
                            ALU.mult, ALU.add)
                    for c in range(NCH):
                        kp = pc_ps.tile([128, SW], F32, tag="pc")
                        qk_mms(kp, wk8_sb, wk16_sb if NF16 else None, c, st)
                        nc.scalar.activation(
                            kt_sb[:, c * CORE_S + st * SW: c * CORE_S + (st + 1) * SW],
                            kp[:], AF.Copy, scale=DSC_K)

                # ---- V projection emitter: first 2 batches upfront, the rest
                # interleaved into the attention stream as PE gap fillers ----
                def emit_v(b, jci, nts=(0, 1)):
                    joff, jlen = JC[jci]
                    vt = vpad[b][jci]
                    if 0 in nts:
                        ones_ap = vt[:jlen].rearrange("p (h c) -> p h c", h=HEADS)[:, :, 64:65]
                        nc.gpsimd.memset(ones_ap, 1.0)
                    scol = b * S + joff
                    for nt, (noff, nlen) in [(n, [(0, 512), (512, 256)][n]) for n in nts]:
                        vp = pc_ps.tile([128, 512], F32, tag="pc",
                                        name=f"vp_{b}_{jci}_{nt}")
                        for c in range(NCH):
                            nc.tensor.matmul(
                                vp[:jlen, :nlen],
                                hs_sb[:, c * CORE_S + scol: c * CORE_S + scol + jlen],
                                wv_sb[:, c * HID + noff: c * HID + noff + nlen],
                                start=(c == 0), stop=(c == NCH - 1))
                        dst = vt[:jlen, nt * 8 * 65:].rearrange(
                            "p (h c) -> p h c", c=65)[:, :nlen // 64, :64]
                        nc.vector.tensor_tensor(
                            out=dst, in0=vp[:jlen, :nlen],
                            in1=bv_sb[:jlen, noff:noff + nlen],
                            op=ALU.add)

                ATTN_ORDER = [6, 7, 0, 1, 2, 3, 4, 5]
                for jci in range(2):
                    emit_v(ATTN_ORDER[0], jci)

                # ---- attention: per batch, two half-groups of 3 head-pairs.
                # Software-pipelined: scores/exp/mul for pair p+1 are emitted
                # before ctx matmuls of pair p so the PE never sits on the
                # exp->mul chain. ctx for 3 pairs accumulates into one
                # 1-bank PSUM tile [128, 390]; normalization is one wide
                # broadcast multiply per (half, i-chunk).
                for bk, b in enumerate(ATTN_ORDER):
                    nxt = ATTN_ORDER[bk + 1] if bk + 1 < NB else None
                    ot = [ot_pool.tile([128, HID], F16, tag="ot",
                                       name=f"ot_{b}_{i}") for i in range(2)]
                    for half in range(2):
                        cps = [pc_ps.tile([128, 390], F32, tag="pc",
                                          name=f"cp_{b}_{half}_{i}") for i in range(2)]

                        def emit_front(hp):
                            ets = [None, None]
                            c = hp
                            col = c * CORE_S + b * S
                            for h in range(2):
                                g = hp * 2 + h
                                sp = sc_ps.tile([128, 2 * S], F32, tag="sp",
                                                name=f"sp_{b}_{hp}_{h}")
                                for jci in range(2):
                                    # jc1 reads a full 128-wide K slice (59 cols of
                                    # next-batch keys); those rows are zeroed by the
                                    # exp(bias) table so the math is unaffected.
                                    nc.tensor.matmul(
                                        sp[:, jci * S:(jci + 1) * S],
                                        kt_sb[h * 64:(h + 1) * 64,
                                              col + jci * 128: col + jci * 128 + 128],
                                        qt_sb[h * 64:(h + 1) * 64, col: col + S],
                                        start=True, stop=True)
                                er = et_pool.tile([128, 2 * S], F16, tag="et",
                                                  name=f"er_{b}_{hp}_{h}")
                                nc.scalar.activation(er[:], sp[:], AF.Exp)
                                et = em_pool.tile([128, 2 * S], F16, tag="em",
                                                  name=f"em_{b}_{hp}_{h}")
                                mul_eng = nc.gpsimd if h == 1 else nc.vector
                                mul_eng.tensor_tensor(
                                    out=et[:], in0=er[:],
                                    in1=eb_sb[:, g * 2 * S:(g + 1) * 2 * S],
                                    op=ALU.mult)
                                ets[h] = et
                            return ets

                        def emit_ctx(hpl, ets):
                            for ici, (ioff, ilen) in enumerate(JC):
                                for h in range(2):
                                    for jci, (joff, jlen) in enumerate(JC):
                                        nc.tensor.matmul(
                                            cps[ici][:ilen, hpl * 130 + h * 65:
                                                     hpl * 130 + (h + 1) * 65],
                                            ets[h][:jlen, jci * S + ioff: jci * S + ioff + ilen],
                                            vpad[b][jci][:jlen,
                                                         ((half * 3 + hpl) * 2 + h) * 65:
                                                         ((half * 3 + hpl) * 2 + h + 1) * 65],
                                            start=(jci == 0), stop=(jci == 1))

                        prev = None
                        for hpl in range(3):
                            ets = emit_front(half * 3 + hpl)
                            if hpl == 1 and nxt is not None:
                                emit_v(nxt, half)
                            if prev is not None:
                                emit_ctx(prev[0], prev[1])
                            prev = (hpl, ets)
                        emit_ctx(prev[0], prev[1])

                        for ici, (ioff, ilen) in enumerate(JC):
                            r = rt_pool.tile([128, 6], F32, tag="rt",
                                             name=f"r_{b}_{half}_{ici}")
                            sums = cps[ici][:ilen].rearrange(
                                "p (g c) -> p g c", c=65)[:, :, 64:65]
                            nc.vector.reciprocal(r[:ilen], sums)
                            nc.vector.tensor_tensor(
                                out=ot[ici][:ilen, half * 384:(half + 1) * 384]
                                    .rearrange("p (g c) -> p g c", c=64),
                                in0=cps[ici][:ilen].rearrange(
                                    "p (g c) -> p g c", c=65)[:, :, :64],
                                in1=r[:ilen].broadcast_to([ilen, 6, 64]),
                                op=ALU.mult)
                            out_eng = nc.sync if (half + ici) % 2 == 0 else nc.scalar
                            out_eng.dma_start(
                                y_d[b, ioff:ioff + ilen, half * 384:(half + 1) * 384],
                                ot[ici][:ilen, half * 384:(half + 1) * 384])

    nc.compile()
    return nc


_NC_CACHE = {}


def _get_nc(reps=1):
    if reps not in _NC_CACHE:
        _NC_CACHE[reps] = build_nc(reps)
    return _NC_CACHE[reps]


def prep_inputs(hidden_states, Wq, bq, Wk, Wv, bv, bias_table):
    hidden_states = np.asarray(hidden_states, np.float32)
    Wq = np.asarray(Wq, np.float32)
    bq = np.asarray(bq, np.float32)
    Wk = np.asarray(Wk, np.float32)
    Wv = np.asarray(Wv, np.float32)
    bv = np.asarray(bv, np.float32)
    bias_table = np.asarray(bias_table, np.float32)
    E4 = ml_dtypes.float8_e4m3

    # fp8 weight layout: [c(dout), h2, k, (i, m)] — pair i over hid 128-blocks
    def w8_pack(wT):
        # wT [hid, dout] scaled; -> [NCH*N_DR, 128, 256]
        w = (wT * SC_W).astype(E4).astype(np.float32)
        w = w.reshape(N_DR, 2, 128, NCH, 128)       # [h2, i, k, c, m]
        w = w.transpose(3, 0, 2, 1, 4)               # [c, h2, k, i, m]
        return np.ascontiguousarray(w.reshape(NCH * N_DR, 128, 256)).astype(E4)

    def w16_pack(wT):
        # fp16 remainder chunks (hid >= 256*N_DR), scaled by SC_F16
        w = (wT[256 * N_DR:] * SC_F16).astype(np.float16)   # [hid_r, dout]
        w = w.reshape(NF16, 128, NCH, 128).transpose(2, 0, 1, 3)  # [c, f, k, m]
        return np.ascontiguousarray(w.reshape(NCH * NF16, 128, 128))

    wq8 = w8_pack(Wq.T)
    wk8 = w8_pack(Wk.T)
    extra = {}
    if NF16:
        extra["wq16"] = w16_pack(Wq.T)
        extra["wk16"] = w16_pack(Wk.T)
    wvT = np.ascontiguousarray(Wv.T).reshape(NCH, 128, HID).astype(np.float16)
    bqc = (bq / 8.0).astype(np.float32).reshape(NCH, 128, 1)
    bvb = np.ascontiguousarray(np.broadcast_to(bv, (128, HID))).astype(np.float16)

    idx = _relative_position_index(14, 14)
    bias_full = bias_table[idx]              # [S, S, HEADS] (i, j, h)
    biasT = bias_full.transpose(2, 1, 0)     # [h, j, i]
    expb = np.zeros((HEADS, 2, 128, S), np.float32)
    for g in range(HEADS):
        for jci, (joff, jlen) in enumerate(JC):
            expb[g, jci, :jlen, :] = np.exp(biasT[g, joff:joff + jlen, :])
    expb = expb.reshape(HEADS * 2, 128, S).astype(np.float16)

    shared = {"wq8": wq8, "wk8": wk8, "wvT": wvT, "bqc": bqc, "bvb": bvb,
              "expb": expb, **extra}
    in_maps = []
    for cc in range(N_CORES):
        hs_c = hidden_states[cc * NB:(cc + 1) * NB]          # [NB, S, HID]
        hsT = np.ascontiguousarray(hs_c.transpose(2, 0, 1).reshape(HID, CORE_S))
        hs8 = (hsT[:256 * N_DR] * SC_HS).astype(E4)          # [hid8, CORE_S]
        hs8 = hs8.reshape(N_DR, 2, 128, CORE_S).transpose(0, 2, 1, 3)  # [h2,k,i,s]
        hs8 = np.ascontiguousarray(hs8.reshape(N_DR, 128, 2 * CORE_S))
        in_maps.append({"hsT": hsT.reshape(NCH, 128, CORE_S).astype(np.float16),
                        "hs8": hs8, **shared})
    return in_maps


def run(in_maps, reps=1, **kw):
    nc = _get_nc(reps)
    res = run_bass_kernel_spmd(nc, in_maps, core_ids=list(range(N_CORES)), **kw)
    out = np.concatenate([res.results[c]["y"] for c in range(N_CORES)], axis=0)
    return out.astype(np.float32), res


def kernel(hidden_states, Wq, bq, Wk, Wv, bv, bias_table,
           resolution_h=224, resolution_w=224):
    assert int(resolution_h) == 224 and int(resolution_w) == 224, \
        "kernel compiled for 224x224 (window 14x14, S=197)"
    hidden_states = np.asarray(hidden_states)
    assert hidden_states.shape == (B, S, HID), hidden_states.shape
    in_maps = prep_inputs(hidden_states, Wq, bq, Wk, Wv, bv, bias_table)
    return run(in_maps, reps=1)[0]


# revision 10
# speedup vs baseline: 1.1239x; 1.1239x over previous
"""Data2VecVision self-attention Bass kernel for 8 Trainium2 NeuronCores.

Sharding: data-parallel over batch (64 = 8 cores x 8 batches/core).

Per-core design (v3 — fp8 DoubleRow Q/K projections):
  - hidden_states shard transposed on host to hsT [768, 8*197]; two SBUF
    copies: fp16 (V projection) and fp8e4 pair-packed st-major (Q/K).
  - Q/K projections are 3 DoubleRow fp8 matmuls each (K=256 logical per
    pass; measured ~2x fp16 throughput at FD=394). hs scaled x16, Wq/Wk
    x1024 into e4m3; descale folded into the PSUM->SBUF copies: Q on ACT
    (activation Copy, scale=descale/8, bias=bq column), K on DVE
    (tensor_scalar mult).
  - V stays fp16 ([s, d_out] natural layout, ones column for softmax
    sums, bv kept in V via the softmax identity).
  - scores for a head PAIR land in one 2-bank PSUM tile [128, 1024] at
    bank-aligned quadrants (h*512 + jc*197); exp runs per head on the
    contiguous [*, h*512 : h*512+394] region; the exp(bias) multiply is
    ONE strided-AP tensor_tensor per pair ([p, 2, 394] views, gaps
    skipped), split DVE/GpSimd.
  - context for 3 head-pairs accumulates into one 1-bank PSUM tile
    [128, 390]; normalization = DVE reciprocal + broadcast multiply ->
    fp16 staging; y stored fp16 (host casts back to fp32).
  - all input DRAM tensors are partition-major so every DMA moves
    multi-KB contiguous runs per partition; ordered so the first Q
    matmul unblocks after ~0.9 MB.
"""

import numpy as np
import ml_dtypes

import concourse.bacc as bacc
import concourse.mybir as mybir
import concourse.tile as tile
from concourse.bass_utils import run_bass_kernel_spmd

F32 = mybir.dt.float32
F16 = mybir.dt.float16
F8 = mybir.dt.float8e4
AF = mybir.ActivationFunctionType
ALU = mybir.AluOpType
DR = mybir.MatmulPerfMode.DoubleRow

N_CORES = 8
B = 64
NB = B // N_CORES          # batches per core
S = 197
HID = 768
HEADS = 12
D = 64
NHP = HEADS // 2           # head pairs
NCH = HID // 128           # 6 contraction chunks (fp16 view)
N_DR = 3                   # fp8 DoubleRow passes (256 hid dims each)
NST = 4                    # projection s-tiles per core
SW = NB * S // NST         # 394, projection moving width
CORE_S = NB * S            # 1576
JC = [(0, 128), (128, 69)]   # j/i chunk (offset, len)
QW = 1024                  # per-pair scores tile width (2 PSUM banks)

SC_HS = 16.0               # fp8 scale for hidden states
SC_W = 1024.0              # fp8 scale for Wq/Wk
DSC_Q = 1.0 / (SC_HS * SC_W * 8.0)   # descale + 1/sqrt(64)
DSC_K = 1.0 / (SC_HS * SC_W)


def _relative_position_index(h, w):
    coords = np.stack(np.meshgrid(np.arange(h), np.arange(w), indexing="ij")).reshape(2, -1)
    rel = coords[:, :, None] - coords[:, None, :]
    rel = rel.transpose(1, 2, 0).astype(np.int64)
    rel[:, :, 0] += h - 1
    rel[:, :, 1] += w - 1
    rel[:, :, 0] *= 2 * w - 1
    area = h * w
    nrd = (2 * h - 1) * (2 * w - 1) + 3
    idx = np.zeros((area + 1, area + 1), dtype=np.int64)
    idx[1:, 1:] = rel.sum(-1)
    idx[0, :] = nrd - 3
    idx[:, 0] = nrd - 2
    idx[0, 0] = nrd - 1
    return idx


def build_nc(reps=1):
    nc = bacc.Bacc("TRN2", target_bir_lowering=False, debug=False)

    SWB = N_DR * 2 * SW                      # 2364, hs8 bytes/partition per st
    hsT_d = nc.dram_tensor("hsT", [NCH, 128, CORE_S], F16, kind="ExternalInput").ap()
    hs8_d = nc.dram_tensor("hs8", [NST, 128, SWB], F8, kind="ExternalInput").ap()
    wq8_d = nc.dram_tensor("wq8", [128, NCH * N_DR * 256], F8, kind="ExternalInput").ap()
    wk8_d = nc.dram_tensor("wk8", [128, NCH * N_DR * 256], F8, kind="ExternalInput").ap()
    wv_d = nc.dram_tensor("wvT", [NCH, 128, HID], F16, kind="ExternalInput").ap()
    bq_d = nc.dram_tensor("bqc", [128, NCH], F32, kind="ExternalInput").ap()
    bv_d = nc.dram_tensor("bvb", [128, HID], F16, kind="ExternalInput").ap()
    eb_d = nc.dram_tensor("expb", [128, NHP * QW], F16, kind="ExternalInput").ap()
    y_d = nc.dram_tensor("y", [NB, S, HID], F16, kind="ExternalOutput").ap()

    with tile.TileContext(nc) as tc:
        with (
            tc.tile_pool(name="res", bufs=1) as res,
            tc.tile_pool(name="vpad", bufs=NB * 2) as vpad_pool,
            tc.tile_pool(name="et", bufs=6) as et_pool,
            tc.tile_pool(name="em", bufs=6) as em_pool,
            tc.tile_pool(name="rt", bufs=6) as rt_pool,
            tc.tile_pool(name="ot", bufs=6) as ot_pool,
            tc.tile_pool(name="pc", bufs=4, space="PSUM") as pc_ps,
            tc.tile_pool(name="sp", bufs=2, space="PSUM") as sc_ps,
        ):
            hs_sb = res.tile([128, NCH * CORE_S], F16)
            hs8_sb = res.tile([128, NST * SWB], F8)
            wq8_sb = res.tile([128, NCH * N_DR * 256], F8)
            wk8_sb = res.tile([128, NCH * N_DR * 256], F8)
            wv_sb = res.tile([128, NCH * HID], F16)
            bq_sb = res.tile([128, NCH], F32)
            bv_sb = res.tile([128, HID], F16)
            eb_sb = res.tile([128, NHP * QW], F16)
            qt_sb = res.tile([128, NCH * CORE_S], F16)
            kt_sb = res.tile([128, NCH * CORE_S + 64], F16)
            nc.vector.memset(kt_sb[:, NCH * CORE_S:], 0.0)
            vpad = [[vpad_pool.tile([128, HEADS * 65], F16, tag="vp",
                                    name=f"vpad_{b}_{j}") for j in range(2)]
                    for b in range(NB)]

            # moving-operand view of hs8: [p, st, h2, two, s]
            hs8_v = hs8_sb.rearrange("p (st h2 two s) -> p st h2 two s",
                                     st=NST, h2=N_DR, two=2)

            for _ in range(reps):
                # ---- input DMAs (partition-major contiguous; ordered so the
                # first Q matmuls unblock early) ----
                dma_engs = [nc.sync, nc.scalar, nc.gpsimd]
                def dma(i, dst, src):
                    dma_engs[i % 3].dma_start(dst, src)
                dma(0, wq8_sb[:, :N_DR * 256], wq8_d[:, :N_DR * 256])
                dma(1, hs8_sb[:, 0:SWB], hs8_d[0])
                dma(2, bq_sb[:], bq_d[:])
                dma(1, wq8_sb[:, N_DR * 256:], wq8_d[:, N_DR * 256:])
                dma(2, wk8_sb[:], wk8_d[:])
                for st in range(1, NST):
                    dma(st, hs8_sb[:, st * SWB:(st + 1) * SWB], hs8_d[st])
                # V inputs: first batches of attention order (6,7) first
                for c in range(NCH):
                    dma(c, hs_sb[:, c * CORE_S + 6 * S: (c + 1) * CORE_S],
                        hsT_d[c, :, 6 * S:])
                dma(0, wv_sb.rearrange("p (x e) -> p x e", e=HID),
                    wv_d.rearrange("x p e -> p x e"))
                dma(1, bv_sb[:], bv_d[:])
                for c in range(NCH):
                    dma(c + 1, hs_sb[:, c * CORE_S: c * CORE_S + 6 * S],
                        hsT_d[c, :, : 6 * S])
                dma(0, eb_sb[:], eb_d[:])

                # ---- Q/K projections, whole core ----
                def qk_mms(dst_ps, w8_sb, c, st):
                    for h2 in range(N_DR):
                        nc.tensor.matmul(
                            dst_ps[:],
                            w8_sb[:, (c * N_DR + h2) * 256:(c * N_DR + h2 + 1) * 256]
                            .rearrange("p (two m) -> p two m", two=2),
                            hs8_v[:, st, h2],
                            start=(h2 == 0), stop=(h2 == N_DR - 1),
                            perf_mode=DR)

                for st in range(NST):
                    for c in range(NCH):
                        qp = pc_ps.tile([128, SW], F32, tag="pc")
                        qk_mms(qp, wq8_sb, c, st)
                        nc.scalar.activation(
                            qt_sb[:, c * CORE_S + st * SW: c * CORE_S + (st + 1) * SW],
                            qp[:], AF.Identity, bias=bq_sb[:, c:c + 1], scale=DSC_Q)
                    for c in range(NCH):
                        kp = pc_ps.tile([128, SW], F32, tag="pc")
                        qk_mms(kp, wk8_sb, c, st)
                        nc.vector.tensor_scalar_mul(
                            kt_sb[:, c * CORE_S + st * SW: c * CORE_S + (st + 1) * SW],
                            kp[:], DSC_K)

                # ---- V projection emitter ----
                def emit_v(b, jci, nts=(0, 1)):
                    joff, jlen = JC[jci]
                    vt = vpad[b][jci]
                    if 0 in nts:
                        ones_ap = vt[:jlen].rearrange("p (h c) -> p h c", h=HEADS)[:, :, 64:65]
                        nc.gpsimd.memset(ones_ap, 1.0)
                    scol = b * S + joff
                    for nt, (noff, nlen) in [(n, [(0, 512), (512, 256)][n]) for n in nts]:
                        vp = pc_ps.tile([128, 512], F32, tag="pc",
                                        name=f"vp_{b}_{jci}_{nt}")
                        for c in range(NCH):
                            nc.tensor.matmul(
                                vp[:jlen, :nlen],
                                hs_sb[:, c * CORE_S + scol: c * CORE_S + scol + jlen],
                                wv_sb[:, c * HID + noff: c * HID + noff + nlen],
                                start=(c == 0), stop=(c == NCH - 1))
                        dst = vt[:jlen, nt * 8 * 65:].rearrange(
                            "p (h c) -> p h c", c=65)[:, :nlen // 64, :64]
                        nc.vector.tensor_tensor(
                            out=dst, in0=vp[:jlen, :nlen],
                            in1=bv_sb[:jlen, noff:noff + nlen],
                            op=ALU.add)

                ATTN_ORDER = [6, 7, 0, 1, 2, 3, 4, 5]
                for jci in range(2):
                    emit_v(ATTN_ORDER[0], jci)

                # ---- attention ----
                for bk, b in enumerate(ATTN_ORDER):
                    nxt = ATTN_ORDER[bk + 1] if bk + 1 < NB else None
                    ot = [ot_pool.tile([128, HID], F16, tag="ot",
                                       name=f"ot_{b}_{i}") for i in range(2)]
                    for half in range(2):
                        cps = [pc_ps.tile([128, 390], F32, tag="pc",
                                          name=f"cp_{b}_{half}_{i}") for i in range(2)]

                        def emit_front(hp):
                            col = hp * CORE_S + b * S
                            sp = sc_ps.tile([128, QW], F32, tag="sp",
                                            name=f"sp_{b}_{hp}")
                            er = et_pool.tile([128, QW], F16, tag="et",
                                              name=f"er_{b}_{hp}")
                            for h in range(2):
                                for jci in range(2):
                                    # jc1 reads a full 128-wide K slice (59 cols
                                    # of next-batch keys); those rows are zeroed
                                    # by the exp(bias) table.
                                    nc.tensor.matmul(
                                        sp[:, h * 512 + jci * S:
                                           h * 512 + (jci + 1) * S],
                                        kt_sb[h * 64:(h + 1) * 64,
                                              col + jci * 128: col + jci * 128 + 128],
                                        qt_sb[h * 64:(h + 1) * 64, col: col + S],
                                        start=True, stop=True)
                                nc.scalar.activation(
                                    er[:, h * 512: h * 512 + 2 * S],
                                    sp[:, h * 512: h * 512 + 2 * S], AF.Exp)
                            et = em_pool.tile([128, QW], F16, tag="em",
                                              name=f"em_{b}_{hp}")
                            for h in range(2):
                                mul_eng = nc.gpsimd if h == 1 else nc.vector
                                mul_eng.tensor_tensor(
                                    out=et[:, h * 512: h * 512 + 2 * S],
                                    in0=er[:, h * 512: h * 512 + 2 * S],
                                    in1=eb_sb[:, hp * QW + h * 512:
                                              hp * QW + h * 512 + 2 * S],
                                    op=ALU.mult)
                            return et

                        def emit_ctx(hpl, et):
                            for ici, (ioff, ilen) in enumerate(JC):
                                for h in range(2):
                                    for jci, (joff, jlen) in enumerate(JC):
                                        nc.tensor.matmul(
                                            cps[ici][:ilen, hpl * 130 + h * 65:
                                                     hpl * 130 + (h + 1) * 65],
                                            et[:jlen, h * 512 + jci * S + ioff:
                                               h * 512 + jci * S + ioff + ilen],
                                            vpad[b][jci][:jlen,
                                                         ((half * 3 + hpl) * 2 + h) * 65:
                                                         ((half * 3 + hpl) * 2 + h + 1) * 65],
                                            start=(jci == 0), stop=(jci == 1))

                        prev = None
                        for hpl in range(3):
                            et = emit_front(half * 3 + hpl)
                            if hpl == 1 and nxt is not None:
                                emit_v(nxt, half)
                            if prev is not None:
                                emit_ctx(prev[0], prev[1])
                            prev = (hpl, et)
                        emit_ctx(prev[0], prev[1])

                        for ici, (ioff, ilen) in enumerate(JC):
                            r = rt_pool.tile([128, 6], F32, tag="rt",
                                             name=f"r_{b}_{half}_{ici}")
                            sums = cps[ici][:ilen].rearrange(
                                "p (g c) -> p g c", c=65)[:, :, 64:65]
                            nc.vector.reciprocal(r[:ilen], sums)
                            nc.vector.tensor_tensor(
                                out=ot[ici][:ilen, half * 384:(half + 1) * 384]
                                    .rearrange("p (g c) -> p g c", c=64),
                                in0=cps[ici][:ilen].rearrange(
                                    "p (g c) -> p g c", c=65)[:, :, :64],
                                in1=r[:ilen].broadcast_to([ilen, 6, 64]),
                                op=ALU.mult)
                            nc.sync.dma_start(
                                y_d[b, ioff:ioff + ilen, half * 384:(half + 1) * 384],
                                ot[ici][:ilen, half * 384:(half + 1) * 384])

    nc.compile()
    return nc


_NC_CACHE = {}


def _get_nc(reps=1):
    if reps not in _NC_CACHE:
        _NC_CACHE[reps] = build_nc(reps)
    return _NC_CACHE[reps]


def prep_inputs(hidden_states, Wq, bq, Wk, Wv, bv, bias_table):
    hidden_states = np.asarray(hidden_states, np.float32)
    Wq = np.asarray(Wq, np.float32)
    bq = np.asarray(bq, np.float32)
    Wk = np.asarray(Wk, np.float32)
    Wv = np.asarray(Wv, np.float32)
    bv = np.asarray(bv, np.float32)
    bias_table = np.asarray(bias_table, np.float32)
    E4 = ml_dtypes.float8_e4m3

    def w8_pack(wT):
        # wT [hid, dout] -> partition-major [128, (c h2 i m)]
        w = (wT * SC_W).astype(E4).astype(np.float32)
        w = w.reshape(N_DR, 2, 128, NCH, 128)       # [h2, i, k, c, m]
        w = w.transpose(2, 3, 0, 1, 4)               # [k, c, h2, i, m]
        return np.ascontiguousarray(w.reshape(128, NCH * N_DR * 256)).astype(E4)

    wq8 = w8_pack(Wq.T)
    wk8 = w8_pack(Wk.T)
    wvT = np.ascontiguousarray(Wv.T).reshape(NCH, 128, HID).astype(np.float16)
    bqc = np.ascontiguousarray((bq / 8.0).astype(np.float32).reshape(NCH, 128).T)
    bvb = np.ascontiguousarray(np.broadcast_to(bv, (128, HID))).astype(np.float16)

    idx = _relative_position_index(14, 14)
    bias_full = bias_table[idx]              # [S, S, HEADS] (i, j, h)
    biasT = bias_full.transpose(2, 1, 0)     # [h, j, i]
    # per-pair exp(bias) table [128, NHP*QW]: pair p, head h, jc quadrant at
    # h*512 + jc*197; gaps zero
    expb = np.zeros((128, NHP, QW), np.float32)
    for hp in range(NHP):
        for h in range(2):
            for jci, (joff, jlen) in enumerate(JC):
                expb[:jlen, hp, h * 512 + jci * S: h * 512 + (jci + 1) * S] = \
                    np.exp(biasT[2 * hp + h, joff:joff + jlen, :])
    expb = np.ascontiguousarray(expb.reshape(128, NHP * QW)).astype(np.float16)

    shared = {"wq8": wq8, "wk8": wk8, "wvT": wvT, "bqc": bqc, "bvb": bvb,
              "expb": expb}
    in_maps = []
    for cc in range(N_CORES):
        hs_c = hidden_states[cc * NB:(cc + 1) * NB]          # [NB, S, HID]
        hsT = np.ascontiguousarray(hs_c.transpose(2, 0, 1).reshape(HID, CORE_S))
        hs8 = (hsT * SC_HS).astype(E4)                        # [hid, CORE_S]
        # -> st-major [NST, 128(k), N_DR(h2), 2(i), SW]
        hs8 = hs8.reshape(N_DR, 2, 128, NST, SW).transpose(3, 2, 0, 1, 4)
        hs8 = np.ascontiguousarray(hs8.reshape(NST, 128, N_DR * 2 * SW))
        in_maps.append({"hsT": hsT.reshape(NCH, 128, CORE_S).astype(np.float16),
                        "hs8": hs8, **shared})
    return in_maps


def run(in_maps, reps=1, **kw):
    nc = _get_nc(reps)
    res = run_bass_kernel_spmd(nc, in_maps, core_ids=list(range(N_CORES)), **kw)
    out = np.concatenate([res.results[c]["y"] for c in range(N_CORES)], axis=0)
    return out.astype(np.float32), res


def kernel(hidden_states, Wq, bq, Wk, Wv, bv, bias_table,
           resolution_h=224, resolution_w=224):
    assert int(resolution_h) == 224 and int(resolution_w) == 224, \
        "kernel compiled for 224x224 (window 14x14, S=197)"
    hidden_states = np.asarray(hidden_states)
    assert hidden_states.shape == (B, S, HID), hidden_states.shape
    in_maps = prep_inputs(hidden_states, Wq, bq, Wk, Wv, bv, bias_table)
    return run(in_maps, reps=1)[0]
